# revision 2
# baseline (speedup 1.0000x reference)
"""Auditory spectrogram kernel for Trainium2 (8 NeuronCores, Bass/Tile), v2.

Pipeline per the reference:
  y1 = order-4 IIR cochlear filterbank (129 channels, per-channel B/A) over wav [8, 64000]
  y2 = sigmoid(y1); y2 = 1st-order IIR (beta) over time
  y4 = relu(y2[c] - y2[c-1]); y5 = 1st-order IIR (alpha); downsample every 256 -> [8, 129, 250]

v2 strategy (vs the fp16 baseline): every matmul runs as an fp8e4m3 DoubleRow
(K=256) at 0.5 cycles/row.
  - Time is re-blocked on a 1-sample-SHIFTED grid: block m holds samples
    128m + p - 127, so each output frame t=256f is the LAST sample of block 2f
    and the alpha-integration becomes a pure per-block weighted reduction
    (no leftover current-sample term). Only blocks 0..498 are needed (the last
    used sample is 63744).
  - S1: per (channel, batch) ONE DoubleRow matmul contracts both 128-tap bands
    (k-tiles read x blocks m-1, m); a second DoubleRow adds the fp8 lo-residual
    of the weights (w = hi + lo gives ~fp16 weight accuracy). x is fp8 with
    first-order noise-shaped quantization (error pushed to high frequencies,
    which the beta-LPF + alpha-integrator downstream attenuate ~100x).
  - Hair-cell nonlinearity stored symmetrically: s' = tanh(y1/2) = 2*sigmoid-1
    (fp8 is 4x more accurate near 0 than near 1); the 0.5 factor is folded into
    the S2 weights.  Channel diff is FUSED into S2: two DoubleRow matmuls per
    (ch,bs) apply (+T1,+T0) to s'_c and (-T1,-T0) to s'_{c-1}; no DVE subtract.
  - S3: one DoubleRow matmul per (ch,bs): an indicator-column stationary
    (alpha^128*w | w zero-padded window trick) reduces y4 block-pairs
    (2f-1, 2f) straight into row r of a shared [128,250] psum; then a single
    tensor_tensor_scan applies the alpha^256 frame recurrence.
  - relu ops are split between DVE and GpSimd to balance engine load;
    sigmoid/tanh stays on the Activation engine.
Sharding: 128 output channels, 16 per core + 1 halo channel (same as baseline).
"""

import numpy as np
import ml_dtypes

NCH, BS, T = 129, 8, 64000
L = 128                      # time block
NBLK = 499                   # shifted-grid blocks (block m: samples 128m+p-127)
SEG = NBLK + 1               # per-batch x/s/y4 segment width (col 0 = zero block)
NFRM = 250
NCORE = 8
CPC = 16
KTAPS = 256                  # FIR truncation (2 bands)
BETA = float(np.exp(-1.0 / 8.0))
ALPHA = float(np.exp(-1.0 / 128.0))
A128 = float(ALPHA ** 128)
A256 = float(ALPHA ** 256)
F8 = ml_dtypes.float8_e4m3fn

_cache = {}


def _impulse_responses(coch_B, coch_A):
    B = coch_B.astype(np.float64)
    A = coch_A.astype(np.float64)
    h = np.zeros((NCH, KTAPS))
    for t in range(KTAPS):
        acc = B[:, t].copy() if t < 5 else np.zeros(NCH)
        for k in range(1, 5):
            if t - k >= 0:
                acc -= A[:, k] * h[:, t - k]
        h[:, t] = acc
    return h


def _nsq8(x):
    """First-order noise-shaped fp8e4m3 quantization along the last axis."""
    x = np.asarray(x, np.float32)
    out = np.empty(x.shape, F8)
    e = np.zeros(x.shape[:-1], np.float32)
    for t in range(x.shape[-1]):
        v = x[..., t] + e
        qv = v.astype(F8)
        out[..., t] = qv
        e = v - qv.astype(np.float32)
    return out


def _band(hc, b):
    p = np.arange(L)
    idx = 128 * b + p[None, :] - p[:, None]
    return np.where(idx >= 0, hc[np.clip(idx, 0, KTAPS - 1)], 0.0)


def _host_prep(wavData, coch_B, coch_A):
    wav = np.asarray(wavData, np.float32)
    h = _impulse_responses(np.asarray(coch_B), np.asarray(coch_A))

    # x: noise-shaped fp8, shifted-grid blocks, interleaved duplicated layout:
    # per batch 2*NBLK cols, col 2m+i = block m-1+i (so the DoubleRow k-tile
    # pair for output block m reads cols (2m, 2m+1), non-overlapping AP)
    xq = _nsq8(wav).astype(np.float32)                     # [8, T]
    wpad = np.zeros((BS, L * NBLK), np.float32)
    n = min(L * NBLK - 127, T)
    wpad[:, 127:127 + n] = xq[:, :n]
    xblk = wpad.reshape(BS, NBLK, L).transpose(2, 0, 1)    # [p, bs, m]
    xt = np.zeros((L, BS, NBLK, 2), np.float32)
    xt[:, :, 1:, 0] = xblk[:, :, :-1]                      # block m-1
    xt[:, :, :, 1] = xblk                                  # block m
    x8 = np.ascontiguousarray(xt.reshape(L, BS * 2 * NBLK)).astype(F8)

    # S1 stationaries per core: [128, 17*512] fp8
    # pos i: cols i*512+[0:128]=W1hi [128:256]=W0hi [256:384]=W1lo [384:512]=W0lo
    W0 = np.stack([_band(h[c], 0) for c in range(NCH)])
    W1 = np.stack([_band(h[c], 1) for c in range(NCH)])
    W0hi = W0.astype(F8)
    W0lo = (W0 - W0hi.astype(np.float64)).astype(F8)
    W1hi = W1.astype(F8)
    W1lo = (W1 - W1hi.astype(np.float64)).astype(F8)
    w1s = []
    for k in range(NCORE):
        W = np.zeros((L, 17 * 512), F8)
        for i in range(CPC + 1):
            c = CPC * k + i
            W[:, i * 512 + 0:i * 512 + 128] = W1hi[c]
            W[:, i * 512 + 128:i * 512 + 256] = W0hi[c]
            W[:, i * 512 + 256:i * 512 + 384] = W1lo[c]
            W[:, i * 512 + 384:i * 512 + 512] = W0lo[c]
        w1s.append(W)

    # S2 stationaries: channel-diff fused via k-tile pairs over the
    # (s_prev, s_cur) segments: M1 ktiles (-T0 | +T0) read block m of both,
    # M2 ktiles (-T1 | +T1) read block m-1 of both.
    p = np.arange(L)
    T0 = np.where(p[None, :] >= p[:, None],
                  BETA ** (p[None, :] - p[:, None]), 0.0) * 0.5
    T1 = np.where(p[:, None] > p[None, :],
                  BETA ** (128 + p[None, :] - p[:, None]), 0.0) * 0.5
    T0q = T0.astype(F8).astype(np.float32)
    T1q = T1.astype(F8).astype(np.float32)
    t2 = np.zeros((L, 512), np.float32)
    t2[:, 0:128] = -T0q
    t2[:, 128:256] = T0q
    t2[:, 256:384] = -T1q
    t2[:, 384:512] = T1q
    t2 = t2.astype(F8)

    # S3 stationary [128, 512]: ktile0 col128 = A128*w, ktile1 col128 = w
    w = ALPHA ** (127 - p)
    wq = np.zeros((L, 512), np.float32)
    wq[:, 128] = A128 * w
    wq[:, 256 + 128] = w
    wq = wq.astype(F8)

    return x8, w1s, t2, wq


def _build(cfg=None, nrep=1, dyn_rep=1, debug=False):
    import contextlib
    import concourse.bacc as bacc
    import concourse.tile as tile
    from concourse import mybir
    from concourse.bass import AP

    f8, f32 = mybir.dt.float8e4, mybir.dt.float32
    DR = mybir.MatmulPerfMode.DoubleRow
    Tanh = mybir.ActivationFunctionType.Tanh

    nc = bacc.Bacc("TRN2", target_bir_lowering=False, debug=False,
                   num_devices=NCORE)
    x_d = nc.dram_tensor("x", [L, BS * 2 * NBLK], f8, kind="ExternalInput")
    w1_d = nc.dram_tensor("w1", [L, 17 * 512], f8, kind="ExternalInput")
    t2_d = nc.dram_tensor("t2", [L, 512], f8, kind="ExternalInput")
    wq_d = nc.dram_tensor("wq", [L, 512], f8, kind="ExternalInput")
    out_d = nc.dram_tensor("out", [L, NFRM], f32, kind="ExternalOutput")
    if debug:
        sdump_d = nc.dram_tensor("sdump", [L, 17 * BS * SEG], f8,
                                 kind="ExternalOutput")
        qdump_d = nc.dram_tensor("qdump", [L, NFRM], f32,
                                 kind="ExternalOutput")
        zdump_d = nc.dram_tensor("zdump", [L, NBLK], f32,
                                 kind="ExternalOutput")
        ydump_d = nc.dram_tensor("ydump", [L, SEG], f8,
                                 kind="ExternalOutput")

    SSEG = BS * SEG            # s cols per position (8 batch segments)
    with tile.TileContext(nc) as tc:
        with tc.tile_pool(name="const", bufs=1) as cp, \
             tc.tile_pool(name="ps1", bufs=2, space="PSUM") as ps1p, \
             tc.tile_pool(name="ps2", bufs=3, space="PSUM") as ps2p, \
             tc.tile_pool(name="psq", bufs=1, space="PSUM") as psqp:
            x_sb = cp.tile([L, BS * 2 * NBLK], f8, name="x_sb")
            w1_sb = cp.tile([L, 17 * 512], f8, name="w1_sb")
            t2_sb = cp.tile([L, 512], f8, name="t2_sb")
            wq_sb = cp.tile([L, 512], f8, name="wq_sb")
            acst = cp.tile([L, NFRM], f32, name="acst")
            f_sb = cp.tile([L, NFRM], f32, name="f_sb")
            s_sb = cp.tile([L, 17 * SSEG], f8, name="s_sb")
            y4_ts = [cp.tile([L, SEG], f8, name=f"y4_{j}") for j in range(16)]

            nc.sync.dma_start(x_sb[:], x_d.ap())
            nc.sync.dma_start(w1_sb[:], w1_d.ap())
            nc.sync.dma_start(t2_sb[:], t2_d.ap())
            nc.sync.dma_start(wq_sb[:], wq_d.ap())
            nc.vector.memset(acst[:], A256)
            nc.vector.memset(s_sb[:, 0:17 * SSEG:SEG], 0.0)
            for yt in y4_ts:
                nc.vector.memset(yt[:, 0:1], 0.0)

            XW, SW, W1W = BS * 2 * NBLK, 17 * SSEG, 17 * 512

            def s1_w(i, lo):
                off = i * 512 + (256 if lo else 0)
                return AP(w1_sb[:, 0:1].tensor, w1_sb[:, 0:1].offset + off,
                          [[W1W, L], [128, 2], [1, 128]])

            def x_mov(bs):
                a = x_sb[:, 0:1]
                return AP(a.tensor, a.offset + bs * 2 * NBLK,
                          [[XW, L], [1, 2], [2, NBLK]])

            def s_mov(i, bs, t1):
                # ktile pair = (s_prev, s_cur) segments; t1 reads block m-1
                a = s_sb[:, 0:1]
                base = (i - 1) * SSEG + bs * SEG + (0 if t1 else 1)
                return AP(a.tensor, a.offset + base,
                          [[SW, L], [SSEG, 2], [1, NBLK]])

            def t2_w(t1):
                a = t2_sb[:, 0:1]
                return AP(a.tensor, a.offset + (256 if t1 else 0),
                          [[512, L], [128, 2], [1, 128]])

            def y4_mov(yt):
                a = yt[:, 0:1]
                return AP(a.tensor, a.offset, [[SEG, L], [1, 2], [2, NFRM]])

            def wq_w(r):
                a = wq_sb[:, 0:1]
                m = 128 if r == 0 else (r + 1)
                return AP(a.tensor, a.offset + 128 - r,
                          [[512, L], [256, 2], [1, m]])

            loop_ctx = (tc.For_i(0, dyn_rep, 1) if dyn_rep > 1
                        else contextlib.nullcontext())
            with loop_ctx:
              for rep in range(nrep):
                psum_q = psqp.tile([L, NFRM], f32, name=f"q_{rep}", tag="q")

                def emit_s1(i):
                    for g in range(4):
                        bsa, bsb = 2 * g, 2 * g + 1
                        pa = ps1p.tile([L, NBLK], f32,
                                       name=f"s1_{rep}_{i}_{bsa}", tag="s1a")
                        pb = ps1p.tile([L, NBLK], f32,
                                       name=f"s1_{rep}_{i}_{bsb}", tag="s1b")
                        for lo in (False, True):
                            wap = s1_w(i, lo)
                            nc.tensor.matmul(pa[:, :], wap, x_mov(bsa),
                                             start=not lo, stop=lo,
                                             perf_mode=DR)
                            nc.tensor.matmul(pb[:, :], wap, x_mov(bsb),
                                             start=not lo, stop=lo,
                                             perf_mode=DR)
                        for bs, ps in ((bsa, pa), (bsb, pb)):
                            nc.scalar.activation(
                                s_sb[:, i * SSEG + bs * SEG + 1:
                                     i * SSEG + (bs + 1) * SEG],
                                ps[:, :], Tanh, scale=0.5)

                def emit_s2(i):
                    # position i >= 1: channel diff fused, 3-batch groups
                    for g0 in range(0, BS, 3):
                        grp = range(g0, min(g0 + 3, BS))
                        pp = {bs: ps2p.tile([L, NBLK], f32,
                                            name=f"s2_{rep}_{i}_{bs}",
                                            tag="s2")
                              for bs in grp}
                        for t1 in (False, True):
                            wap = t2_w(t1)
                            for bs in grp:
                                nc.tensor.matmul(pp[bs][:, :], wap,
                                                 s_mov(i, bs, t1),
                                                 start=not t1, stop=t1,
                                                 perf_mode=DR)
                        for bs in grp:
                            r = (i - 1) * BS + bs
                            yt = y4_ts[r % 16]
                            if debug and r == 0:
                                zd = cp.tile([L, NBLK], f32, name="zd")
                                nc.scalar.copy(zd[:], pp[bs][:, :])
                                nc.sync.dma_start(zdump_d.ap(), zd[:])
                            if r % 8 == 7:
                                nc.scalar.activation(
                                    yt[:, 1:SEG], pp[bs][:, :],
                                    mybir.ActivationFunctionType.Relu)
                            else:
                                nc.vector.tensor_scalar_max(
                                    yt[:, 1:SEG], pp[bs][:, :], 0.0)
                            if debug and r == 0:
                                nc.sync.dma_start(ydump_d.ap(), yt[:])

                def emit_q(i):
                    for bs in range(BS):
                        r = (i - 1) * BS + bs
                        yt = y4_ts[r % 16]
                        out_ap = (psum_q[:, :] if r == 0
                                  else psum_q[0:r + 1, :])
                        nc.tensor.matmul(out_ap, wq_w(r), y4_mov(yt),
                                         start=(r == 0), stop=(r == 127),
                                         perf_mode=DR)

                # software-pipelined emission: S1(i) | S2(i-1) | q(i-2)
                for i in range(19):
                    if i <= 16:
                        emit_s1(i)
                    if 1 <= i - 1 <= 16:
                        emit_s2(i - 1)
                    if 1 <= i - 2 <= 16:
                        emit_q(i - 2)

                if debug:
                    qd = cp.tile([L, NFRM], f32, name=f"qd_{rep}")
                    nc.scalar.copy(qd[:], psum_q[:])
                    nc.sync.dma_start(qdump_d.ap(), qd[:])
                    nc.sync.dma_start(sdump_d.ap(), s_sb[:])
                nc.vector.tensor_tensor_scan(
                    f_sb[:], acst[:], psum_q[:],
                    0.0, mybir.AluOpType.mult, mybir.AluOpType.add)
                nc.sync.dma_start(out_d.ap(), f_sb[:])
    _dedupe_ldweights(nc)
    nc.compile()
    return nc


def _dedupe_ldweights(nc):
    """Drop PE weight loads whose stationary operand matches the previous
    load in the scheduled PE stream (the splitter emits one per matmul)."""
    from concourse import mybir
    dropped = 0
    for bb in nc.m.functions[0].blocks:
        last_key = None
        keep = []
        for inst in bb.instructions:
            if isinstance(inst, mybir.InstLdweights):
                si = inst.sync_info
                a = inst.ins[0]
                key = (str(a.ap), a.offset, str(a.dtype), str(a.memref),
                       str(getattr(inst, "perf_mode", None)))
                if (key == last_key and not (si and (si.on_wait or si.on_update))):
                    dropped += 1
                    continue
                last_key = key
            elif isinstance(inst, (mybir.InstUnconditionalBranch,
                                   mybir.InstCompareAndBranch)):
                last_key = None
            keep.append(inst)
        if len(keep) != len(bb.instructions):
            bb.instructions = keep
    return dropped


def _make_runner(nc):
    """Persistent jitted 8-core runner (mirrors bass2jax.run_bass_via_pjrt)."""
    import jax
    from jax.sharding import Mesh, PartitionSpec
    from jax.experimental.shard_map import shard_map
    from concourse import bass2jax, mybir

    bass2jax.install_neuronx_cc_hook()

    partition_name = (
        nc.partition_id_tensor.name if nc.partition_id_tensor else None
    )
    in_names, out_names, out_avals, zero_shapes = [], [], [], []
    for alloc in nc.m.functions[0].allocations:
        if not isinstance(alloc, mybir.MemoryLocationSet):
            continue
        name = alloc.memorylocations[0].name
        if alloc.kind == "ExternalInput":
            if name != partition_name:
                in_names.append(name)
        elif alloc.kind == "ExternalOutput":
            out_names.append(name)
            shape = tuple(alloc.tensor_shape)
            dtype = mybir.dt.np(alloc.dtype)
            out_avals.append(jax.core.ShapedArray(shape, dtype))
            zero_shapes.append((shape, dtype))
    n_params = len(in_names)
    all_in_names = list(in_names) + list(out_names)
    if partition_name is not None:
        all_in_names.append(partition_name)

    def _body(*args):
        operands = list(args)
        if partition_name is not None:
            operands.append(bass2jax.partition_id_tensor())
        outs = bass2jax._bass_exec_p.bind(
            *operands,
            out_avals=tuple(out_avals),
            in_names=tuple(all_in_names),
            out_names=tuple(out_names),
            lowering_input_output_aliases=(),
            sim_require_finite=True,
            sim_require_nnan=True,
            nc=nc,
        )
        return tuple(outs)

    devices = jax.devices()[:NCORE]
    mesh = Mesh(np.asarray(devices), ("core",))
    n_outs = len(out_names)
    sharded = jax.jit(
        shard_map(_body, mesh=mesh,
                  in_specs=(PartitionSpec("core"),) * (n_params + n_outs),
                  out_specs=(PartitionSpec("core"),) * n_outs,
                  check_rep=False),
        donate_argnums=tuple(range(n_params, n_params + n_outs)),
        keep_unused=True,
    )

    def run(in_maps):
        concat_in = [
            np.concatenate([np.asarray(m[name]) for m in in_maps], axis=0)
            for name in in_names
        ]
        concat_zeros = [
            np.zeros((NCORE * s[0], *s[1:]), d) for (s, d) in zero_shapes
        ]
        out_arrs = sharded(*concat_in, *concat_zeros)
        return [
            {name: np.asarray(out_arrs[i]).reshape(NCORE, *out_avals[i].shape)[c]
             for i, name in enumerate(out_names)}
            for c in range(NCORE)
        ]

    return run


def make_in_maps(prep):
    x8, w1s, t2, wq = prep
    return [dict(x=x8, w1=w1s[k], t2=t2, wq=wq) for k in range(NCORE)]


def _get_runner(wavData, coch_B, coch_A):
    prep = _host_prep(wavData, coch_B, coch_A)
    if "v2" not in _cache:
        nc = _build()
        _cache["v2"] = _make_runner(nc)
    return _cache["v2"], make_in_maps(prep)


def kernel(wavData, coch_B, coch_A):
    run, in_maps = _get_runner(wavData, coch_B, coch_A)
    results = run(in_maps)
    out = np.zeros((BS, NCH, NFRM), np.float32)
    for k in range(NCORE):
        F = results[k]["out"]                      # [128, 250]
        out[:, CPC * k + 1: CPC * (k + 1) + 1, :] = \
            F.reshape(CPC, BS, NFRM).transpose(1, 0, 2)
    return out


# revision 3
# speedup vs baseline: 1.2390x; 1.2390x over previous
"""Auditory spectrogram kernel for Trainium2 (8 NeuronCores, Bass/Tile), v2.

Pipeline per the reference:
  y1 = order-4 IIR cochlear filterbank (129 channels, per-channel B/A) over wav [8, 64000]
  y2 = sigmoid(y1); y2 = 1st-order IIR (beta) over time
  y4 = relu(y2[c] - y2[c-1]); y5 = 1st-order IIR (alpha); downsample every 256 -> [8, 129, 250]

v2 strategy (vs the fp16 baseline): every matmul runs as an fp8e4m3 DoubleRow
(K=256) at 0.5 cycles/row.
  - Time is re-blocked on a 1-sample-SHIFTED grid: block m holds samples
    128m + p - 127, so each output frame t=256f is the LAST sample of block 2f
    and the alpha-integration becomes a pure per-block weighted reduction
    (no leftover current-sample term). Only blocks 0..498 are needed (the last
    used sample is 63744).
  - S1: per (channel, batch) ONE DoubleRow matmul contracts both 128-tap bands
    (k-tiles read x blocks m-1, m); a second DoubleRow adds the fp8 lo-residual
    of the weights (w = hi + lo gives ~fp16 weight accuracy). x is fp8 with
    first-order noise-shaped quantization (error pushed to high frequencies,
    which the beta-LPF + alpha-integrator downstream attenuate ~100x).
  - Hair-cell nonlinearity stored symmetrically: s' = tanh(y1/2) = 2*sigmoid-1
    (fp8 is 4x more accurate near 0 than near 1); the 0.5 factor is folded into
    the S2 weights.  Channel diff is FUSED into S2: two DoubleRow matmuls per
    (ch,bs) apply (+T1,+T0) to s'_c and (-T1,-T0) to s'_{c-1}; no DVE subtract.
  - S3: one DoubleRow matmul per (ch,bs): an indicator-column stationary
    (alpha^128*w | w zero-padded window trick) reduces y4 block-pairs
    (2f-1, 2f) straight into row r of a shared [128,250] psum; then a single
    tensor_tensor_scan applies the alpha^256 frame recurrence.
  - relu ops are split between DVE and GpSimd to balance engine load;
    sigmoid/tanh stays on the Activation engine.
Sharding: 128 output channels, 16 per core + 1 halo channel (same as baseline).
"""

import numpy as np
import ml_dtypes

NCH, BS, T = 129, 8, 64000
L = 128                      # time block
NBLK = 499                   # shifted-grid blocks (block m: samples 128m+p-127)
SEG = NBLK + 1               # per-batch x/s/y4 segment width (col 0 = zero block)
NFRM = 250
NCORE = 8
CPC = 16
KTAPS = 256                  # FIR truncation (2 bands)
BETA = float(np.exp(-1.0 / 8.0))
ALPHA = float(np.exp(-1.0 / 128.0))
A128 = float(ALPHA ** 128)
A256 = float(ALPHA ** 256)
F8 = ml_dtypes.float8_e4m3fn

_cache = {}


def _impulse_responses(coch_B, coch_A):
    B = coch_B.astype(np.float64)
    A = coch_A.astype(np.float64)
    h = np.zeros((NCH, KTAPS))
    for t in range(KTAPS):
        acc = B[:, t].copy() if t < 5 else np.zeros(NCH)
        for k in range(1, 5):
            if t - k >= 0:
                acc -= A[:, k] * h[:, t - k]
        h[:, t] = acc
    return h


def _nsq8(x):
    """First-order noise-shaped fp8e4m3 quantization along the last axis."""
    x = np.asarray(x, np.float32)
    out = np.empty(x.shape, F8)
    e = np.zeros(x.shape[:-1], np.float32)
    for t in range(x.shape[-1]):
        v = x[..., t] + e
        qv = v.astype(F8)
        out[..., t] = qv
        e = v - qv.astype(np.float32)
    return out


def _band(hc, b):
    p = np.arange(L)
    idx = 128 * b + p[None, :] - p[:, None]
    return np.where(idx >= 0, hc[np.clip(idx, 0, KTAPS - 1)], 0.0)


def _host_prep(wavData, coch_B, coch_A):
    wav = np.asarray(wavData, np.float32)
    h = _impulse_responses(np.asarray(coch_B), np.asarray(coch_A))

    # x: noise-shaped fp8, shifted-grid blocks, interleaved duplicated layout:
    # per batch 2*NBLK cols, col 2m+i = block m-1+i (so the DoubleRow k-tile
    # pair for output block m reads cols (2m, 2m+1), non-overlapping AP)
    xq = _nsq8(wav).astype(np.float32)                     # [8, T]
    wpad = np.zeros((BS, L * NBLK), np.float32)
    n = min(L * NBLK - 127, T)
    wpad[:, 127:127 + n] = xq[:, :n]
    xblk = wpad.reshape(BS, NBLK, L).transpose(2, 0, 1)    # [p, bs, m]
    xt = np.zeros((L, BS, NBLK, 2), np.float32)
    xt[:, :, 1:, 0] = xblk[:, :, :-1]                      # block m-1
    xt[:, :, :, 1] = xblk                                  # block m
    x8 = np.ascontiguousarray(xt.reshape(L, BS * 2 * NBLK)).astype(F8)

    # S1 stationaries per core: [128, 17*512] fp8
    # pos i: cols i*512+[0:128]=W1hi [128:256]=W0hi [256:384]=W1lo [384:512]=W0lo
    W0 = np.stack([_band(h[c], 0) for c in range(NCH)])
    W1 = np.stack([_band(h[c], 1) for c in range(NCH)])
    W0hi = W0.astype(F8)
    W0lo = (W0 - W0hi.astype(np.float64)).astype(F8)
    W1hi = W1.astype(F8)
    W1lo = (W1 - W1hi.astype(np.float64)).astype(F8)
    w1s = []
    for k in range(NCORE):
        W = np.zeros((L, 17 * 512), F8)
        for i in range(CPC + 1):
            c = CPC * k + i
            W[:, i * 512 + 0:i * 512 + 128] = W1hi[c]
            W[:, i * 512 + 128:i * 512 + 256] = W0hi[c]
            W[:, i * 512 + 256:i * 512 + 384] = W1lo[c]
            W[:, i * 512 + 384:i * 512 + 512] = W0lo[c]
        w1s.append(W)

    # S2 stationaries: channel-diff fused via k-tile pairs over the
    # (s_prev, s_cur) segments: M1 ktiles (-T0 | +T0) read block m of both,
    # M2 ktiles (-T1 | +T1) read block m-1 of both.
    p = np.arange(L)
    T0 = np.where(p[None, :] >= p[:, None],
                  BETA ** (p[None, :] - p[:, None]), 0.0) * 0.5
    T1 = np.where(p[:, None] > p[None, :],
                  BETA ** (128 + p[None, :] - p[:, None]), 0.0) * 0.5
    T0q = T0.astype(F8).astype(np.float32)
    T1q = T1.astype(F8).astype(np.float32)
    t2 = np.zeros((L, 512), np.float32)
    t2[:, 0:128] = -T0q
    t2[:, 128:256] = T0q
    t2[:, 256:384] = -T1q
    t2[:, 384:512] = T1q
    t2 = t2.astype(F8)

    # S3 stationary [128, 512]: ktile0 col128 = A128*w, ktile1 col128 = w
    w = ALPHA ** (127 - p)
    wq = np.zeros((L, 512), np.float32)
    wq[:, 128] = A128 * w
    wq[:, 256 + 128] = w
    wq = wq.astype(F8)

    return x8, w1s, t2, wq


def _build(cfg=None, nrep=1, dyn_rep=1, debug=False):
    import contextlib
    import concourse.bacc as bacc
    import concourse.tile as tile
    from concourse import mybir
    from concourse.bass import AP

    f8, f32 = mybir.dt.float8e4, mybir.dt.float32
    DR = mybir.MatmulPerfMode.DoubleRow
    Tanh = mybir.ActivationFunctionType.Tanh

    nc = bacc.Bacc("TRN2", target_bir_lowering=False, debug=False,
                   num_devices=NCORE)
    x_d = nc.dram_tensor("x", [L, BS * 2 * NBLK], f8, kind="ExternalInput")
    w1_d = nc.dram_tensor("w1", [L, 17 * 512], f8, kind="ExternalInput")
    t2_d = nc.dram_tensor("t2", [L, 512], f8, kind="ExternalInput")
    wq_d = nc.dram_tensor("wq", [L, 512], f8, kind="ExternalInput")
    out_d = nc.dram_tensor("out", [L, NFRM], f32, kind="ExternalOutput")
    if debug:
        sdump_d = nc.dram_tensor("sdump", [L, 17 * BS * SEG], f8,
                                 kind="ExternalOutput")
        qdump_d = nc.dram_tensor("qdump", [L, NFRM], f32,
                                 kind="ExternalOutput")
        zdump_d = nc.dram_tensor("zdump", [L, NBLK], f32,
                                 kind="ExternalOutput")
        ydump_d = nc.dram_tensor("ydump", [L, SEG], f8,
                                 kind="ExternalOutput")

    SSEG = BS * SEG            # s cols per position (8 batch segments)
    with tile.TileContext(nc) as tc:
        with tc.tile_pool(name="const", bufs=1) as cp, \
             tc.tile_pool(name="ps1", bufs=2, space="PSUM") as ps1p, \
             tc.tile_pool(name="ps2", bufs=3, space="PSUM") as ps2p, \
             tc.tile_pool(name="psq", bufs=1, space="PSUM") as psqp:
            x_sb = cp.tile([L, BS * 2 * NBLK], f8, name="x_sb")
            w1_sb = cp.tile([L, 17 * 512], f8, name="w1_sb")
            t2_sb = cp.tile([L, 512], f8, name="t2_sb")
            wq_sb = cp.tile([L, 512], f8, name="wq_sb")
            acst = cp.tile([L, NFRM], f32, name="acst")
            f_sb = cp.tile([L, NFRM], f32, name="f_sb")
            s_sb = cp.tile([L, 17 * SSEG], f8, name="s_sb")
            y4_ts = [cp.tile([L, SEG], f8, name=f"y4_{j}") for j in range(16)]

            nc.sync.dma_start(x_sb[:], x_d.ap())
            nc.sync.dma_start(w1_sb[:], w1_d.ap())
            nc.sync.dma_start(t2_sb[:], t2_d.ap())
            nc.sync.dma_start(wq_sb[:], wq_d.ap())
            nc.vector.memset(acst[:], A256)
            nc.vector.memset(s_sb[:, 0:17 * SSEG:SEG], 0.0)
            for yt in y4_ts:
                nc.vector.memset(yt[:, 0:1], 0.0)

            XW, SW, W1W = BS * 2 * NBLK, 17 * SSEG, 17 * 512

            def s1_w(i, lo):
                off = i * 512 + (256 if lo else 0)
                return AP(w1_sb[:, 0:1].tensor, w1_sb[:, 0:1].offset + off,
                          [[W1W, L], [128, 2], [1, 128]])

            def x_mov(bs):
                a = x_sb[:, 0:1]
                return AP(a.tensor, a.offset + bs * 2 * NBLK,
                          [[XW, L], [1, 2], [2, NBLK]])

            def s_mov(i, bs, t1):
                # ktile pair = (s_prev, s_cur) segments; t1 reads block m-1
                a = s_sb[:, 0:1]
                base = (i - 1) * SSEG + bs * SEG + (0 if t1 else 1)
                return AP(a.tensor, a.offset + base,
                          [[SW, L], [SSEG, 2], [1, NBLK]])

            def t2_w(t1):
                a = t2_sb[:, 0:1]
                return AP(a.tensor, a.offset + (256 if t1 else 0),
                          [[512, L], [128, 2], [1, 128]])

            def y4_mov(yt):
                a = yt[:, 0:1]
                return AP(a.tensor, a.offset, [[SEG, L], [1, 2], [2, NFRM]])

            def wq_w(r):
                a = wq_sb[:, 0:1]
                m = 128 if r == 0 else (r + 1)
                return AP(a.tensor, a.offset + 128 - r,
                          [[512, L], [256, 2], [1, m]])

            loop_ctx = (tc.For_i(0, dyn_rep, 1) if dyn_rep > 1
                        else contextlib.nullcontext())
            with loop_ctx:
              for rep in range(nrep):
                psum_q = psqp.tile([L, NFRM], f32, name=f"q_{rep}", tag="q")

                def emit_s1(i):
                    for g in range(4):
                        bsa, bsb = 2 * g, 2 * g + 1
                        pa = ps1p.tile([L, NBLK], f32,
                                       name=f"s1_{rep}_{i}_{bsa}", tag="s1a")
                        pb = ps1p.tile([L, NBLK], f32,
                                       name=f"s1_{rep}_{i}_{bsb}", tag="s1b")
                        for lo in (False, True):
                            wap = s1_w(i, lo)
                            nc.tensor.matmul(pa[:, :], wap, x_mov(bsa),
                                             start=not lo, stop=lo,
                                             perf_mode=DR)
                            nc.tensor.matmul(pb[:, :], wap, x_mov(bsb),
                                             start=not lo, stop=lo,
                                             perf_mode=DR)
                        for bs, ps in ((bsa, pa), (bsb, pb)):
                            nc.scalar.activation(
                                s_sb[:, i * SSEG + bs * SEG + 1:
                                     i * SSEG + (bs + 1) * SEG],
                                ps[:, :], Tanh, scale=0.5)

                def emit_s2(i):
                    # position i >= 1: channel diff fused, 3-batch groups
                    for g0 in range(0, BS, 3):
                        grp = range(g0, min(g0 + 3, BS))
                        pp = {bs: ps2p.tile([L, NBLK], f32,
                                            name=f"s2_{rep}_{i}_{bs}",
                                            tag="s2")
                              for bs in grp}
                        for t1 in (False, True):
                            wap = t2_w(t1)
                            for bs in grp:
                                nc.tensor.matmul(pp[bs][:, :], wap,
                                                 s_mov(i, bs, t1),
                                                 start=not t1, stop=t1,
                                                 perf_mode=DR)
                        for bs in grp:
                            r = (i - 1) * BS + bs
                            yt = y4_ts[r % 16]
                            if debug and r == 0:
                                zd = cp.tile([L, NBLK], f32, name="zd")
                                nc.scalar.copy(zd[:], pp[bs][:, :])
                                nc.sync.dma_start(zdump_d.ap(), zd[:])
                            if r % 16 == 15:
                                nc.scalar.activation(
                                    yt[:, 1:SEG], pp[bs][:, :],
                                    mybir.ActivationFunctionType.Relu)
                            else:
                                nc.vector.tensor_scalar_max(
                                    yt[:, 1:SEG], pp[bs][:, :], 0.0)
                            if debug and r == 0:
                                nc.sync.dma_start(ydump_d.ap(), yt[:])

                def emit_q(i):
                    for bs in range(BS):
                        r = (i - 1) * BS + bs
                        yt = y4_ts[r % 16]
                        out_ap = (psum_q[:, :] if r == 0
                                  else psum_q[0:r + 1, :])
                        nc.tensor.matmul(out_ap, wq_w(r), y4_mov(yt),
                                         start=(r == 0), stop=(r == 127),
                                         perf_mode=DR)

                # software-pipelined emission: S1(i) | S2(i-1) | q(i-2)
                for i in range(19):
                    if i <= 16:
                        emit_s1(i)
                    if 1 <= i - 1 <= 16:
                        emit_s2(i - 1)
                    if 1 <= i - 2 <= 16:
                        emit_q(i - 2)

                if debug:
                    qd = cp.tile([L, NFRM], f32, name=f"qd_{rep}")
                    nc.scalar.copy(qd[:], psum_q[:])
                    nc.sync.dma_start(qdump_d.ap(), qd[:])
                    nc.sync.dma_start(sdump_d.ap(), s_sb[:])
                nc.vector.tensor_tensor_scan(
                    f_sb[:], acst[:], psum_q[:],
                    0.0, mybir.AluOpType.mult, mybir.AluOpType.add)
                nc.sync.dma_start(out_d.ap(), f_sb[:])
    _dedupe_ldweights(nc)
    nc.compile()
    return nc


def _dedupe_ldweights(nc):
    """Drop PE weight loads whose stationary operand matches the previous
    load in the scheduled PE stream (the splitter emits one per matmul)."""
    from concourse import mybir
    dropped = 0
    for bb in nc.m.functions[0].blocks:
        last_key = None
        keep = []
        for inst in bb.instructions:
            if isinstance(inst, mybir.InstLdweights):
                si = inst.sync_info
                a = inst.ins[0]
                key = (str(a.ap), a.offset, str(a.dtype), str(a.memref),
                       str(getattr(inst, "perf_mode", None)))
                if (key == last_key and not (si and (si.on_wait or si.on_update))):
                    dropped += 1
                    continue
                last_key = key
            elif isinstance(inst, (mybir.InstUnconditionalBranch,
                                   mybir.InstCompareAndBranch)):
                last_key = None
            keep.append(inst)
        if len(keep) != len(bb.instructions):
            bb.instructions = keep
    return dropped


def _make_runner(nc):
    """Persistent jitted 8-core runner (mirrors bass2jax.run_bass_via_pjrt)."""
    import jax
    from jax.sharding import Mesh, PartitionSpec
    from jax.experimental.shard_map import shard_map
    from concourse import bass2jax, mybir

    bass2jax.install_neuronx_cc_hook()

    partition_name = (
        nc.partition_id_tensor.name if nc.partition_id_tensor else None
    )
    in_names, out_names, out_avals, zero_shapes = [], [], [], []
    for alloc in nc.m.functions[0].allocations:
        if not isinstance(alloc, mybir.MemoryLocationSet):
            continue
        name = alloc.memorylocations[0].name
        if alloc.kind == "ExternalInput":
            if name != partition_name:
                in_names.append(name)
        elif alloc.kind == "ExternalOutput":
            out_names.append(name)
            shape = tuple(alloc.tensor_shape)
            dtype = mybir.dt.np(alloc.dtype)
            out_avals.append(jax.core.ShapedArray(shape, dtype))
            zero_shapes.append((shape, dtype))
    n_params = len(in_names)
    all_in_names = list(in_names) + list(out_names)
    if partition_name is not None:
        all_in_names.append(partition_name)

    def _body(*args):
        operands = list(args)
        if partition_name is not None:
            operands.append(bass2jax.partition_id_tensor())
        outs = bass2jax._bass_exec_p.bind(
            *operands,
            out_avals=tuple(out_avals),
            in_names=tuple(all_in_names),
            out_names=tuple(out_names),
            lowering_input_output_aliases=(),
            sim_require_finite=True,
            sim_require_nnan=True,
            nc=nc,
        )
        return tuple(outs)

    devices = jax.devices()[:NCORE]
    mesh = Mesh(np.asarray(devices), ("core",))
    n_outs = len(out_names)
    sharded = jax.jit(
        shard_map(_body, mesh=mesh,
                  in_specs=(PartitionSpec("core"),) * (n_params + n_outs),
                  out_specs=(PartitionSpec("core"),) * n_outs,
                  check_rep=False),
        donate_argnums=tuple(range(n_params, n_params + n_outs)),
        keep_unused=True,
    )

    def run(in_maps):
        concat_in = [
            np.concatenate([np.asarray(m[name]) for m in in_maps], axis=0)
            for name in in_names
        ]
        concat_zeros = [
            np.zeros((NCORE * s[0], *s[1:]), d) for (s, d) in zero_shapes
        ]
        out_arrs = sharded(*concat_in, *concat_zeros)
        return [
            {name: np.asarray(out_arrs[i]).reshape(NCORE, *out_avals[i].shape)[c]
             for i, name in enumerate(out_names)}
            for c in range(NCORE)
        ]

    return run


def make_in_maps(prep):
    x8, w1s, t2, wq = prep
    return [dict(x=x8, w1=w1s[k], t2=t2, wq=wq) for k in range(NCORE)]


def _get_runner(wavData, coch_B, coch_A):
    prep = _host_prep(wavData, coch_B, coch_A)
    if "v2" not in _cache:
        nc = _build()
        _cache["v2"] = _make_runner(nc)
    return _cache["v2"], make_in_maps(prep)


def kernel(wavData, coch_B, coch_A):
    run, in_maps = _get_runner(wavData, coch_B, coch_A)
    results = run(in_maps)
    out = np.zeros((BS, NCH, NFRM), np.float32)
    for k in range(NCORE):
        F = results[k]["out"]                      # [128, 250]
        out[:, CPC * k + 1: CPC * (k + 1) + 1, :] = \
            F.reshape(CPC, BS, NFRM).transpose(1, 0, 2)
    return out


# revision 5
# speedup vs baseline: 1.2407x; 1.0014x over previous
"""Auditory spectrogram kernel for Trainium2 (8 NeuronCores, Bass/Tile), v2.

Pipeline per the reference:
  y1 = order-4 IIR cochlear filterbank (129 channels, per-channel B/A) over wav [8, 64000]
  y2 = sigmoid(y1); y2 = 1st-order IIR (beta) over time
  y4 = relu(y2[c] - y2[c-1]); y5 = 1st-order IIR (alpha); downsample every 256 -> [8, 129, 250]

v2 strategy (vs the fp16 baseline): every matmul runs as an fp8e4m3 DoubleRow
(K=256) at 0.5 cycles/row.
  - Time is re-blocked on a 1-sample-SHIFTED grid: block m holds samples
    128m + p - 127, so each output frame t=256f is the LAST sample of block 2f
    and the alpha-integration becomes a pure per-block weighted reduction
    (no leftover current-sample term). Only blocks 0..498 are needed (the last
    used sample is 63744).
  - S1: per (channel, batch) ONE DoubleRow matmul contracts both 128-tap bands
    (k-tiles read x blocks m-1, m); a second DoubleRow adds the fp8 lo-residual
    of the weights (w = hi + lo gives ~fp16 weight accuracy). x is fp8 with
    first-order noise-shaped quantization (error pushed to high frequencies,
    which the beta-LPF + alpha-integrator downstream attenuate ~100x).
  - Hair-cell nonlinearity stored symmetrically: s' = tanh(y1/2) = 2*sigmoid-1
    (fp8 is 4x more accurate near 0 than near 1); the 0.5 factor is folded into
    the S2 weights.  Channel diff is FUSED into S2: two DoubleRow matmuls per
    (ch,bs) apply (+T1,+T0) to s'_c and (-T1,-T0) to s'_{c-1}; no DVE subtract.
  - S3: one DoubleRow matmul per (ch,bs): an indicator-column stationary
    (alpha^128*w | w zero-padded window trick) reduces y4 block-pairs
    (2f-1, 2f) straight into row r of a shared [128,250] psum; then a single
    tensor_tensor_scan applies the alpha^256 frame recurrence.
  - relu ops run on DVE (1 in 16 on Act) to balance engine load;
    sigmoid/tanh stays on the Activation engine (GpSimd has no PSUM port).
Sharding: 128 output channels, 16 per core + 1 halo channel (same as baseline).
"""

import numpy as np
import ml_dtypes

NCH, BS, T = 129, 8, 64000
L = 128                      # time block
NBLK = 499                   # shifted-grid blocks (block m: samples 128m+p-127)
SEG = NBLK + 1               # per-batch x/s/y4 segment width (col 0 = zero block)
NFRM = 250
NCORE = 8
CPC = 16
KTAPS = 256                  # FIR truncation (2 bands)
BETA = float(np.exp(-1.0 / 8.0))
ALPHA = float(np.exp(-1.0 / 128.0))
A128 = float(ALPHA ** 128)
A256 = float(ALPHA ** 256)
F8 = ml_dtypes.float8_e4m3fn

_cache = {}


def _impulse_responses(coch_B, coch_A):
    B = coch_B.astype(np.float64)
    A = coch_A.astype(np.float64)
    h = np.zeros((NCH, KTAPS))
    for t in range(KTAPS):
        acc = B[:, t].copy() if t < 5 else np.zeros(NCH)
        for k in range(1, 5):
            if t - k >= 0:
                acc -= A[:, k] * h[:, t - k]
        h[:, t] = acc
    return h


def _nsq8(x):
    """First-order noise-shaped fp8e4m3 quantization along the last axis."""
    x = np.asarray(x, np.float32)
    out = np.empty(x.shape, F8)
    e = np.zeros(x.shape[:-1], np.float32)
    for t in range(x.shape[-1]):
        v = x[..., t] + e
        qv = v.astype(F8)
        out[..., t] = qv
        e = v - qv.astype(np.float32)
    return out


def _band(hc, b):
    p = np.arange(L)
    idx = 128 * b + p[None, :] - p[:, None]
    return np.where(idx >= 0, hc[np.clip(idx, 0, KTAPS - 1)], 0.0)


def _host_prep(wavData, coch_B, coch_A):
    wav = np.asarray(wavData, np.float32)
    h = _impulse_responses(np.asarray(coch_B), np.asarray(coch_A))

    # x: noise-shaped fp8, shifted-grid blocks, interleaved duplicated layout:
    # per batch 1024 cols (512 block-pairs), col 2m+i = block m-1+i (so the
    # DoubleRow k-tile pair for output block m reads cols (2m, 2m+1)).
    # Pairs m >= 499 are zero so S1 psum cols 499..511 compute to 0, letting
    # one activation span two batches' psum banks with tanh(0)=0 landing on
    # the next segment's zero-pad column.
    xq = _nsq8(wav).astype(np.float32)                     # [8, T]
    wpad = np.zeros((BS, L * NBLK), np.float32)
    n = min(L * NBLK - 127, T)
    wpad[:, 127:127 + n] = xq[:, :n]
    xblk = wpad.reshape(BS, NBLK, L).transpose(2, 0, 1)    # [p, bs, m]
    xt = np.zeros((L, BS, 512, 2), np.float32)
    xt[:, :, 1:NBLK + 1, 0] = xblk                         # block m-1
    xt[:, :, 0:NBLK, 1] = xblk                             # block m
    x8 = np.ascontiguousarray(xt.reshape(L, BS * 1024)).astype(F8)

    # S1 stationaries per core: [128, 17*512] fp8
    # pos i: cols i*512+[0:128]=W1hi [128:256]=W0hi [256:384]=W1lo [384:512]=W0lo
    W0 = np.stack([_band(h[c], 0) for c in range(NCH)])
    W1 = np.stack([_band(h[c], 1) for c in range(NCH)])
    W0hi = W0.astype(F8)
    W0lo = (W0 - W0hi.astype(np.float64)).astype(F8)
    W1hi = W1.astype(F8)
    W1lo = (W1 - W1hi.astype(np.float64)).astype(F8)
    w1s = []
    for k in range(NCORE):
        W = np.zeros((L, 17 * 512), F8)
        for i in range(CPC + 1):
            c = CPC * k + i
            W[:, i * 512 + 0:i * 512 + 128] = W1hi[c]
            W[:, i * 512 + 128:i * 512 + 256] = W0hi[c]
            W[:, i * 512 + 256:i * 512 + 384] = W1lo[c]
            W[:, i * 512 + 384:i * 512 + 512] = W0lo[c]
        w1s.append(W)

    # S2 stationaries: channel-diff fused via k-tile pairs over the
    # (s_prev, s_cur) segments: M1 ktiles (-T0 | +T0) read block m of both,
    # M2 ktiles (-T1 | +T1) read block m-1 of both.
    p = np.arange(L)
    T0 = np.where(p[None, :] >= p[:, None],
                  BETA ** (p[None, :] - p[:, None]), 0.0) * 0.5
    T1 = np.where(p[:, None] > p[None, :],
                  BETA ** (128 + p[None, :] - p[:, None]), 0.0) * 0.5
    T0q = T0.astype(F8).astype(np.float32)
    T1q = T1.astype(F8).astype(np.float32)
    t2 = np.zeros((L, 512), np.float32)
    t2[:, 0:128] = -T0q
    t2[:, 128:256] = T0q
    t2[:, 256:384] = -T1q
    t2[:, 384:512] = T1q
    t2 = t2.astype(F8)

    # S3 stationary [128, 512]: ktile0 col128 = A128*w, ktile1 col128 = w
    w = ALPHA ** (127 - p)
    wq = np.zeros((L, 512), np.float32)
    wq[:, 128] = A128 * w
    wq[:, 256 + 128] = w
    wq = wq.astype(F8)

    return x8, w1s, t2, wq


def _build(cfg=None, nrep=1, dyn_rep=1, debug=False):
    import contextlib
    import concourse.bacc as bacc
    import concourse.tile as tile
    from concourse import mybir
    from concourse.bass import AP

    f8, f32 = mybir.dt.float8e4, mybir.dt.float32
    DR = mybir.MatmulPerfMode.DoubleRow
    Tanh = mybir.ActivationFunctionType.Tanh

    nc = bacc.Bacc("TRN2", target_bir_lowering=False, debug=False,
                   num_devices=NCORE)
    x_d = nc.dram_tensor("x", [L, BS * 1024], f8, kind="ExternalInput")
    w1_d = nc.dram_tensor("w1", [L, 17 * 512], f8, kind="ExternalInput")
    t2_d = nc.dram_tensor("t2", [L, 512], f8, kind="ExternalInput")
    wq_d = nc.dram_tensor("wq", [L, 512], f8, kind="ExternalInput")
    out_d = nc.dram_tensor("out", [L, NFRM], f32, kind="ExternalOutput")
    if debug:
        sdump_d = nc.dram_tensor("sdump", [L, 17 * BS * SEG], f8,
                                 kind="ExternalOutput")
        qdump_d = nc.dram_tensor("qdump", [L, NFRM], f32,
                                 kind="ExternalOutput")
        zdump_d = nc.dram_tensor("zdump", [L, NBLK], f32,
                                 kind="ExternalOutput")
        ydump_d = nc.dram_tensor("ydump", [L, SEG], f8,
                                 kind="ExternalOutput")

    SSEG = BS * 512            # s cols per position (8 batch segments of 512)
    with tile.TileContext(nc) as tc:
        with tc.tile_pool(name="const", bufs=1) as cp, \
             tc.tile_pool(name="ps1", bufs=2, space="PSUM") as ps1p, \
             tc.tile_pool(name="ps2", bufs=3, space="PSUM") as ps2p, \
             tc.tile_pool(name="psq", bufs=1, space="PSUM") as psqp:
            x_sb = cp.tile([L, BS * 1024], f8, name="x_sb")
            w1_sb = cp.tile([L, 17 * 512], f8, name="w1_sb")
            t2_sb = cp.tile([L, 512], f8, name="t2_sb")
            wq_sb = cp.tile([L, 512], f8, name="wq_sb")
            acst = cp.tile([L, NFRM], f32, name="acst")
            f_sb = cp.tile([L, NFRM], f32, name="f_sb")
            s_sb = cp.tile([L, 17 * SSEG], f8, name="s_sb")
            y4_ts = [cp.tile([L, SEG], f8, name=f"y4_{j}") for j in range(16)]

            nc.sync.dma_start(x_sb[:], x_d.ap())
            nc.sync.dma_start(w1_sb[:], w1_d.ap())
            nc.sync.dma_start(t2_sb[:], t2_d.ap())
            nc.sync.dma_start(wq_sb[:], wq_d.ap())
            nc.vector.memset(acst[:], A256)
            nc.vector.memset(s_sb[:, 0:17 * SSEG:512], 0.0)
            for yt in y4_ts:
                nc.vector.memset(yt[:, 0:1], 0.0)

            XW, SW, W1W = BS * 1024, 17 * SSEG, 17 * 512

            def s1_w(i, lo):
                off = i * 512 + (256 if lo else 0)
                return AP(w1_sb[:, 0:1].tensor, w1_sb[:, 0:1].offset + off,
                          [[W1W, L], [128, 2], [1, 128]])

            def x_mov(bs):
                a = x_sb[:, 0:1]
                return AP(a.tensor, a.offset + bs * 1024,
                          [[XW, L], [1, 2], [2, 512]])

            def s_mov(i, bs, t1):
                # ktile pair = (s_prev, s_cur) segments; t1 reads block m-1
                a = s_sb[:, 0:1]
                base = (i - 1) * SSEG + bs * 512 + (0 if t1 else 1)
                return AP(a.tensor, a.offset + base,
                          [[SW, L], [SSEG, 2], [1, NBLK]])

            def t2_w(t1):
                a = t2_sb[:, 0:1]
                return AP(a.tensor, a.offset + (256 if t1 else 0),
                          [[512, L], [128, 2], [1, 128]])

            def y4_mov(yt):
                a = yt[:, 0:1]
                return AP(a.tensor, a.offset, [[SEG, L], [1, 2], [2, NFRM]])

            def wq_w(r):
                a = wq_sb[:, 0:1]
                m = 128 if r == 0 else (r + 1)
                return AP(a.tensor, a.offset + 128 - r,
                          [[512, L], [256, 2], [1, m]])

            loop_ctx = (tc.For_i(0, dyn_rep, 1) if dyn_rep > 1
                        else contextlib.nullcontext())
            with loop_ctx:
              for rep in range(nrep):
                psum_q = psqp.tile([L, NFRM], f32, name=f"q_{rep}", tag="q")

                def emit_s1(i):
                    for g in range(4):
                        bsa, bsb = 2 * g, 2 * g + 1
                        pp = ps1p.tile([L, 1024], f32,
                                       name=f"s1_{rep}_{i}_{g}", tag="s1")
                        for lo in (False, True):
                            wap = s1_w(i, lo)
                            nc.tensor.matmul(pp[:, 0:512], wap, x_mov(bsa),
                                             start=not lo, stop=lo,
                                             perf_mode=DR)
                            nc.tensor.matmul(pp[:, 512:1024], wap, x_mov(bsb),
                                             start=not lo, stop=lo,
                                             perf_mode=DR)
                        # one activation covers both halves; psum cols
                        # 499..511 are exact zeros (zero x pairs), so the
                        # next segment's pad col gets tanh(0) = 0
                        nc.scalar.activation(
                            s_sb[:, i * SSEG + bsa * 512 + 1:
                                 i * SSEG + bsa * 512 + 1012],
                            pp[:, 0:1011], Tanh, scale=0.5)

                def emit_s2(i):
                    # position i >= 1: channel diff fused, 3-batch groups
                    for g0 in range(0, BS, 3):
                        grp = range(g0, min(g0 + 3, BS))
                        pp = {bs: ps2p.tile([L, NBLK], f32,
                                            name=f"s2_{rep}_{i}_{bs}",
                                            tag="s2")
                              for bs in grp}
                        for t1 in (False, True):
                            wap = t2_w(t1)
                            for bs in grp:
                                nc.tensor.matmul(pp[bs][:, :], wap,
                                                 s_mov(i, bs, t1),
                                                 start=not t1, stop=t1,
                                                 perf_mode=DR)
                        for bs in grp:
                            r = (i - 1) * BS + bs
                            yt = y4_ts[r % 16]
                            if debug and r == 0:
                                zd = cp.tile([L, NBLK], f32, name="zd")
                                nc.scalar.copy(zd[:], pp[bs][:, :])
                                nc.sync.dma_start(zdump_d.ap(), zd[:])
                            if r % 16 == 15:
                                nc.scalar.activation(
                                    yt[:, 1:SEG], pp[bs][:, :],
                                    mybir.ActivationFunctionType.Relu)
                            else:
                                nc.vector.tensor_scalar_max(
                                    yt[:, 1:SEG], pp[bs][:, :], 0.0)
                            if debug and r == 0:
                                nc.sync.dma_start(ydump_d.ap(), yt[:])

                def emit_q(i):
                    for bs in range(BS):
                        r = (i - 1) * BS + bs
                        yt = y4_ts[r % 16]
                        out_ap = (psum_q[:, :] if r == 0
                                  else psum_q[0:r + 1, :])
                        nc.tensor.matmul(out_ap, wq_w(r), y4_mov(yt),
                                         start=(r == 0), stop=(r == 127),
                                         perf_mode=DR)

                # software-pipelined emission: S1(i) | S2(i-1) | q(i-2)
                for i in range(19):
                    if i <= 16:
                        emit_s1(i)
                    if 1 <= i - 1 <= 16:
                        emit_s2(i - 1)
                    if 1 <= i - 2 <= 16:
                        emit_q(i - 2)

                if debug:
                    qd = cp.tile([L, NFRM], f32, name=f"qd_{rep}")
                    nc.scalar.copy(qd[:], psum_q[:])
                    nc.sync.dma_start(qdump_d.ap(), qd[:])
                    nc.sync.dma_start(sdump_d.ap(), s_sb[:])
                nc.vector.tensor_tensor_scan(
                    f_sb[:], acst[:], psum_q[:],
                    0.0, mybir.AluOpType.mult, mybir.AluOpType.add)
                nc.sync.dma_start(out_d.ap(), f_sb[:])
    _dedupe_ldweights(nc)
    nc.compile()
    return nc


def _dedupe_ldweights(nc):
    """Drop PE weight loads whose stationary operand matches the previous
    load in the scheduled PE stream (the splitter emits one per matmul)."""
    from concourse import mybir
    dropped = 0
    for bb in nc.m.functions[0].blocks:
        last_key = None
        keep = []
        for inst in bb.instructions:
            if isinstance(inst, mybir.InstLdweights):
                si = inst.sync_info
                a = inst.ins[0]
                key = (str(a.ap), a.offset, str(a.dtype), str(a.memref),
                       str(getattr(inst, "perf_mode", None)))
                if (key == last_key and not (si and (si.on_wait or si.on_update))):
                    dropped += 1
                    continue
                last_key = key
            elif isinstance(inst, (mybir.InstUnconditionalBranch,
                                   mybir.InstCompareAndBranch)):
                last_key = None
            keep.append(inst)
        if len(keep) != len(bb.instructions):
            bb.instructions = keep
    return dropped


def _make_runner(nc):
    """Persistent jitted 8-core runner (mirrors bass2jax.run_bass_via_pjrt)."""
    import jax
    from jax.sharding import Mesh, PartitionSpec
    from jax.experimental.shard_map import shard_map
    from concourse import bass2jax, mybir

    bass2jax.install_neuronx_cc_hook()

    partition_name = (
        nc.partition_id_tensor.name if nc.partition_id_tensor else None
    )
    in_names, out_names, out_avals, zero_shapes = [], [], [], []
    for alloc in nc.m.functions[0].allocations:
        if not isinstance(alloc, mybir.MemoryLocationSet):
            continue
        name = alloc.memorylocations[0].name
        if alloc.kind == "ExternalInput":
            if name != partition_name:
                in_names.append(name)
        elif alloc.kind == "ExternalOutput":
            out_names.append(name)
            shape = tuple(alloc.tensor_shape)
            dtype = mybir.dt.np(alloc.dtype)
            out_avals.append(jax.core.ShapedArray(shape, dtype))
            zero_shapes.append((shape, dtype))
    n_params = len(in_names)
    all_in_names = list(in_names) + list(out_names)
    if partition_name is not None:
        all_in_names.append(partition_name)

    def _body(*args):
        operands = list(args)
        if partition_name is not None:
            operands.append(bass2jax.partition_id_tensor())
        outs = bass2jax._bass_exec_p.bind(
            *operands,
            out_avals=tuple(out_avals),
            in_names=tuple(all_in_names),
            out_names=tuple(out_names),
            lowering_input_output_aliases=(),
            sim_require_finite=True,
            sim_require_nnan=True,
            nc=nc,
        )
        return tuple(outs)

    devices = jax.devices()[:NCORE]
    mesh = Mesh(np.asarray(devices), ("core",))
    n_outs = len(out_names)
    sharded = jax.jit(
        shard_map(_body, mesh=mesh,
                  in_specs=(PartitionSpec("core"),) * (n_params + n_outs),
                  out_specs=(PartitionSpec("core"),) * n_outs,
                  check_rep=False),
        donate_argnums=tuple(range(n_params, n_params + n_outs)),
        keep_unused=True,
    )

    def run(in_maps):
        concat_in = [
            np.concatenate([np.asarray(m[name]) for m in in_maps], axis=0)
            for name in in_names
        ]
        concat_zeros = [
            np.zeros((NCORE * s[0], *s[1:]), d) for (s, d) in zero_shapes
        ]
        out_arrs = sharded(*concat_in, *concat_zeros)
        return [
            {name: np.asarray(out_arrs[i]).reshape(NCORE, *out_avals[i].shape)[c]
             for i, name in enumerate(out_names)}
            for c in range(NCORE)
        ]

    return run


def make_in_maps(prep):
    x8, w1s, t2, wq = prep
    return [dict(x=x8, w1=w1s[k], t2=t2, wq=wq) for k in range(NCORE)]


def _get_runner(wavData, coch_B, coch_A):
    prep = _host_prep(wavData, coch_B, coch_A)
    if "v2" not in _cache:
        nc = _build()
        _cache["v2"] = _make_runner(nc)
    return _cache["v2"], make_in_maps(prep)


def kernel(wavData, coch_B, coch_A):
    run, in_maps = _get_runner(wavData, coch_B, coch_A)
    results = run(in_maps)
    out = np.zeros((BS, NCH, NFRM), np.float32)
    for k in range(NCORE):
        F = results[k]["out"]                      # [128, 250]
        out[:, CPC * k + 1: CPC * (k + 1) + 1, :] = \
            F.reshape(CPC, BS, NFRM).transpose(1, 0, 2)
    return out


# revision 6
# speedup vs baseline: 1.5613x; 1.2584x over previous
"""Auditory spectrogram kernel for Trainium2 (8 NeuronCores, Bass/Tile), v2.

Pipeline per the reference:
  y1 = order-4 IIR cochlear filterbank (129 channels, per-channel B/A) over wav [8, 64000]
  y2 = sigmoid(y1); y2 = 1st-order IIR (beta) over time
  y4 = relu(y2[c] - y2[c-1]); y5 = 1st-order IIR (alpha); downsample every 256 -> [8, 129, 250]

v2 strategy (vs the fp16 baseline): every matmul runs as an fp8e4m3 DoubleRow
(K=256) at 0.5 cycles/row.
  - Time is re-blocked on a 1-sample-SHIFTED grid: block m holds samples
    128m + p - 127, so each output frame t=256f is the LAST sample of block 2f
    and the alpha-integration becomes a pure per-block weighted reduction
    (no leftover current-sample term). Only blocks 0..498 are needed (the last
    used sample is 63744).
  - S1: per (channel, batch) ONE DoubleRow matmul contracts both 128-tap bands
    (k-tiles read x blocks m-1, m); a second DoubleRow adds the fp8 lo-residual
    of the weights (w = hi + lo gives ~fp16 weight accuracy). x is fp8 with
    first-order noise-shaped quantization (error pushed to high frequencies,
    which the beta-LPF + alpha-integrator downstream attenuate ~100x).
  - Hair-cell nonlinearity stored symmetrically: s' = tanh(y1/2) = 2*sigmoid-1
    (fp8 is 4x more accurate near 0 than near 1); the 0.5 factor is folded into
    the S2 weights.  Channel diff is FUSED into S2: two DoubleRow matmuls per
    (ch,bs) apply (+T1,+T0) to s'_c and (-T1,-T0) to s'_{c-1}; no DVE subtract.
  - S3: one DoubleRow matmul per (ch,bs): an indicator-column stationary
    (alpha^128*w | w zero-padded window trick) reduces y4 block-pairs
    (2f-1, 2f) straight into row r of a shared [128,250] psum; then a single
    tensor_tensor_scan applies the alpha^256 frame recurrence.
  - relu ops run on DVE (1 in 16 on Act) to balance engine load;
    sigmoid/tanh stays on the Activation engine (GpSimd has no PSUM port).
Sharding: 128 output channels, 16 per core + 1 halo channel (same as baseline).
"""

import numpy as np
import ml_dtypes

NCH, BS, T = 129, 8, 64000
L = 128                      # time block
NBLK = 499                   # shifted-grid blocks (block m: samples 128m+p-127)
SEG = NBLK + 1               # per-batch x/s/y4 segment width (col 0 = zero block)
NFRM = 250
NCORE = 8
CPC = 16
KTAPS = 256                  # FIR truncation (2 bands)
BETA = float(np.exp(-1.0 / 8.0))
ALPHA = float(np.exp(-1.0 / 128.0))
A128 = float(ALPHA ** 128)
A256 = float(ALPHA ** 256)
F8 = ml_dtypes.float8_e4m3fn

_cache = {}
_LO_KEEP = tuple(range(17))


def _impulse_responses(coch_B, coch_A):
    B = coch_B.astype(np.float64)
    A = coch_A.astype(np.float64)
    h = np.zeros((NCH, KTAPS))
    for t in range(KTAPS):
        acc = B[:, t].copy() if t < 5 else np.zeros(NCH)
        for k in range(1, 5):
            if t - k >= 0:
                acc -= A[:, k] * h[:, t - k]
        h[:, t] = acc
    return h


def _nsq8(x):
    """First-order noise-shaped fp8e4m3 quantization along the last axis."""
    x = np.asarray(x, np.float32)
    out = np.empty(x.shape, F8)
    e = np.zeros(x.shape[:-1], np.float32)
    for t in range(x.shape[-1]):
        v = x[..., t] + e
        qv = v.astype(F8)
        out[..., t] = qv
        e = v - qv.astype(np.float32)
    return out


def _band(hc, b):
    p = np.arange(L)
    idx = 128 * b + p[None, :] - p[:, None]
    return np.where(idx >= 0, hc[np.clip(idx, 0, KTAPS - 1)], 0.0)


def _host_prep(wavData, coch_B, coch_A):
    wav = np.asarray(wavData, np.float32)
    h = _impulse_responses(np.asarray(coch_B), np.asarray(coch_A))

    # x: noise-shaped fp8, shifted-grid blocks, interleaved duplicated layout:
    # per batch 1024 cols (512 block-pairs), col 2m+i = block m-1+i (so the
    # DoubleRow k-tile pair for output block m reads cols (2m, 2m+1)).
    # Pairs m >= 499 are zero so S1 psum cols 499..511 compute to 0, letting
    # one activation span two batches' psum banks with tanh(0)=0 landing on
    # the next segment's zero-pad column.
    xq = _nsq8(wav).astype(np.float32)                     # [8, T]
    wpad = np.zeros((BS, L * NBLK), np.float32)
    n = min(L * NBLK - 127, T)
    wpad[:, 127:127 + n] = xq[:, :n]
    xblk = wpad.reshape(BS, NBLK, L).transpose(2, 0, 1)    # [p, bs, m]
    xt = np.zeros((L, BS, 512, 2), np.float32)
    xt[:, :, 1:NBLK + 1, 0] = xblk                         # block m-1
    xt[:, :, 0:NBLK, 1] = xblk                             # block m
    x8 = np.ascontiguousarray(xt.reshape(L, BS * 1024)).astype(F8)

    # S1 stationaries per core: [128, 17*512] fp8
    # pos i: cols i*512+[0:128]=W1hi [128:256]=W0hi [256:384]=W1lo [384:512]=W0lo
    W0 = np.stack([_band(h[c], 0) for c in range(NCH)])
    W1 = np.stack([_band(h[c], 1) for c in range(NCH)])
    W0hi = W0.astype(F8)
    W0lo = (W0 - W0hi.astype(np.float64)).astype(F8)
    W1hi = W1.astype(F8)
    W1lo = (W1 - W1hi.astype(np.float64)).astype(F8)
    w1s = []
    for k in range(NCORE):
        W = np.zeros((L, 17 * 512), F8)
        for i in range(CPC + 1):
            c = CPC * k + i
            W[:, i * 512 + 0:i * 512 + 128] = W1hi[c]
            W[:, i * 512 + 128:i * 512 + 256] = W0hi[c]
            W[:, i * 512 + 256:i * 512 + 384] = W1lo[c]
            W[:, i * 512 + 384:i * 512 + 512] = W0lo[c]
        w1s.append(W)

    # S2 stationaries: channel-diff fused via k-tile pairs over the
    # (s_prev, s_cur) segments: M1 ktiles (-T0 | +T0) read block m of both,
    # M2 ktiles (-T1 | +T1) read block m-1 of both.
    p = np.arange(L)
    T0 = np.where(p[None, :] >= p[:, None],
                  BETA ** (p[None, :] - p[:, None]), 0.0) * 0.5
    T1 = np.where(p[:, None] > p[None, :],
                  BETA ** (128 + p[None, :] - p[:, None]), 0.0) * 0.5
    T0q = T0.astype(F8).astype(np.float32)
    T1q = T1.astype(F8).astype(np.float32)
    t2 = np.zeros((L, 512), np.float32)
    t2[:, 0:128] = -T0q
    t2[:, 128:256] = T0q
    t2[:, 256:384] = -T1q
    t2[:, 384:512] = T1q
    t2 = t2.astype(F8)

    # S3 stationary [128, 512]: ktile0 col128 = A128*w, ktile1 col128 = w
    w = ALPHA ** (127 - p)
    wq = np.zeros((L, 512), np.float32)
    wq[:, 128] = A128 * w
    wq[:, 256 + 128] = w
    wq = wq.astype(F8)

    # positions whose channels need the lo-residual weight matmul (the rest
    # run hi-only: the w-quant error of low-gain filters is negligible)
    hh = np.concatenate([W0, W1], axis=2).reshape(NCH, -1)
    hhi = np.concatenate([W0hi.astype(np.float64), W1hi.astype(np.float64)],
                         axis=2).reshape(NCH, -1)
    score = np.abs(hh - hhi).sum(1)
    pos_score = np.array([
        max(score[i + CPC * k] for k in range(NCORE) if i + CPC * k < NCH)
        for i in range(CPC + 1)
    ])
    keep = tuple(sorted(np.argsort(pos_score)[::-1][:4].tolist()))
    global _LO_KEEP
    _LO_KEEP = keep
    return x8, w1s, t2, wq


def _build(cfg=None, nrep=1, dyn_rep=1, debug=False):
    import contextlib
    import concourse.bacc as bacc
    import concourse.tile as tile
    from concourse import mybir
    from concourse.bass import AP

    f8, f32 = mybir.dt.float8e4, mybir.dt.float32
    DR = mybir.MatmulPerfMode.DoubleRow
    Tanh = mybir.ActivationFunctionType.Tanh

    nc = bacc.Bacc("TRN2", target_bir_lowering=False, debug=False,
                   num_devices=NCORE)
    x_d = nc.dram_tensor("x", [L, BS * 1024], f8, kind="ExternalInput")
    w1_d = nc.dram_tensor("w1", [L, 17 * 512], f8, kind="ExternalInput")
    t2_d = nc.dram_tensor("t2", [L, 512], f8, kind="ExternalInput")
    wq_d = nc.dram_tensor("wq", [L, 512], f8, kind="ExternalInput")
    out_d = nc.dram_tensor("out", [L, NFRM], f32, kind="ExternalOutput")
    if debug:
        sdump_d = nc.dram_tensor("sdump", [L, 17 * BS * SEG], f8,
                                 kind="ExternalOutput")
        qdump_d = nc.dram_tensor("qdump", [L, NFRM], f32,
                                 kind="ExternalOutput")
        zdump_d = nc.dram_tensor("zdump", [L, NBLK], f32,
                                 kind="ExternalOutput")
        ydump_d = nc.dram_tensor("ydump", [L, SEG], f8,
                                 kind="ExternalOutput")

    SSEG = BS * 512            # s cols per position (8 batch segments of 512)
    with tile.TileContext(nc) as tc:
        with tc.tile_pool(name="const", bufs=1) as cp, \
             tc.tile_pool(name="ps1", bufs=2, space="PSUM") as ps1p, \
             tc.tile_pool(name="ps2", bufs=3, space="PSUM") as ps2p, \
             tc.tile_pool(name="psq", bufs=1, space="PSUM") as psqp:
            x_sb = cp.tile([L, BS * 1024], f8, name="x_sb")
            w1_sb = cp.tile([L, 17 * 512], f8, name="w1_sb")
            t2_sb = cp.tile([L, 512], f8, name="t2_sb")
            wq_sb = cp.tile([L, 512], f8, name="wq_sb")
            acst = cp.tile([L, NFRM], f32, name="acst")
            f_sb = cp.tile([L, NFRM], f32, name="f_sb")
            s_sb = cp.tile([L, 17 * SSEG], f8, name="s_sb")
            y4_ts = [cp.tile([L, SEG], f8, name=f"y4_{j}") for j in range(16)]

            nc.sync.dma_start(x_sb[:], x_d.ap())
            nc.sync.dma_start(w1_sb[:], w1_d.ap())
            nc.sync.dma_start(t2_sb[:], t2_d.ap())
            nc.sync.dma_start(wq_sb[:], wq_d.ap())
            nc.vector.memset(acst[:], A256)
            nc.vector.memset(s_sb[:, 0:17 * SSEG:512], 0.0)
            for yt in y4_ts:
                nc.vector.memset(yt[:, 0:1], 0.0)

            XW, SW, W1W = BS * 1024, 17 * SSEG, 17 * 512

            def s1_w(i, lo):
                off = i * 512 + (256 if lo else 0)
                return AP(w1_sb[:, 0:1].tensor, w1_sb[:, 0:1].offset + off,
                          [[W1W, L], [128, 2], [1, 128]])

            def x_mov(bs):
                a = x_sb[:, 0:1]
                return AP(a.tensor, a.offset + bs * 1024,
                          [[XW, L], [1, 2], [2, 512]])

            def s_mov(i, bs, t1):
                # ktile pair = (s_prev, s_cur) segments; t1 reads block m-1
                a = s_sb[:, 0:1]
                base = (i - 1) * SSEG + bs * 512 + (0 if t1 else 1)
                return AP(a.tensor, a.offset + base,
                          [[SW, L], [SSEG, 2], [1, NBLK]])

            def t2_w(t1):
                a = t2_sb[:, 0:1]
                return AP(a.tensor, a.offset + (256 if t1 else 0),
                          [[512, L], [128, 2], [1, 128]])

            def y4_mov(yt):
                a = yt[:, 0:1]
                return AP(a.tensor, a.offset, [[SEG, L], [1, 2], [2, NFRM]])

            def wq_w(r):
                a = wq_sb[:, 0:1]
                m = 128 if r == 0 else (r + 1)
                return AP(a.tensor, a.offset + 128 - r,
                          [[512, L], [256, 2], [1, m]])

            loop_ctx = (tc.For_i(0, dyn_rep, 1) if dyn_rep > 1
                        else contextlib.nullcontext())
            with loop_ctx:
              for rep in range(nrep):
                psum_q = psqp.tile([L, NFRM], f32, name=f"q_{rep}", tag="q")

                def emit_s1(i):
                    los = (False, True) if i in _LO_KEEP else (False,)
                    for g in range(4):
                        bsa, bsb = 2 * g, 2 * g + 1
                        pp = ps1p.tile([L, 1024], f32,
                                       name=f"s1_{rep}_{i}_{g}", tag="s1")
                        for lo in los:
                            wap = s1_w(i, lo)
                            nc.tensor.matmul(pp[:, 0:512], wap, x_mov(bsa),
                                             start=not lo, stop=(lo == los[-1]),
                                             perf_mode=DR)
                            nc.tensor.matmul(pp[:, 512:1024], wap, x_mov(bsb),
                                             start=not lo, stop=(lo == los[-1]),
                                             perf_mode=DR)
                        # one activation covers both halves; psum cols
                        # 499..511 are exact zeros (zero x pairs), so the
                        # next segment's pad col gets tanh(0) = 0
                        nc.scalar.activation(
                            s_sb[:, i * SSEG + bsa * 512 + 1:
                                 i * SSEG + bsa * 512 + 1012],
                            pp[:, 0:1011], Tanh, scale=0.5)

                def emit_s2(i):
                    # position i >= 1: channel diff fused, 3-batch groups
                    for g0 in range(0, BS, 3):
                        grp = range(g0, min(g0 + 3, BS))
                        pp = {bs: ps2p.tile([L, NBLK], f32,
                                            name=f"s2_{rep}_{i}_{bs}",
                                            tag="s2")
                              for bs in grp}
                        for t1 in (False, True):
                            wap = t2_w(t1)
                            for bs in grp:
                                nc.tensor.matmul(pp[bs][:, :], wap,
                                                 s_mov(i, bs, t1),
                                                 start=not t1, stop=t1,
                                                 perf_mode=DR)
                        for bs in grp:
                            r = (i - 1) * BS + bs
                            yt = y4_ts[r % 16]
                            if debug and r == 0:
                                zd = cp.tile([L, NBLK], f32, name="zd")
                                nc.scalar.copy(zd[:], pp[bs][:, :])
                                nc.sync.dma_start(zdump_d.ap(), zd[:])
                            if r % 16 == 15:
                                nc.scalar.activation(
                                    yt[:, 1:SEG], pp[bs][:, :],
                                    mybir.ActivationFunctionType.Relu)
                            else:
                                nc.vector.tensor_scalar_max(
                                    yt[:, 1:SEG], pp[bs][:, :], 0.0)
                            if debug and r == 0:
                                nc.sync.dma_start(ydump_d.ap(), yt[:])

                def emit_q(i):
                    for bs in range(BS):
                        r = (i - 1) * BS + bs
                        yt = y4_ts[r % 16]
                        out_ap = (psum_q[:, :] if r == 0
                                  else psum_q[0:r + 1, :])
                        nc.tensor.matmul(out_ap, wq_w(r), y4_mov(yt),
                                         start=(r == 0), stop=(r == 127),
                                         perf_mode=DR)

                # software-pipelined emission: S1(i) | S2(i-1) | q(i-2)
                for i in range(19):
                    if i <= 16:
                        emit_s1(i)
                    if 1 <= i - 1 <= 16:
                        emit_s2(i - 1)
                    if 1 <= i - 2 <= 16:
                        emit_q(i - 2)

                if debug:
                    qd = cp.tile([L, NFRM], f32, name=f"qd_{rep}")
                    nc.scalar.copy(qd[:], psum_q[:])
                    nc.sync.dma_start(qdump_d.ap(), qd[:])
                    nc.sync.dma_start(sdump_d.ap(), s_sb[:])
                nc.vector.tensor_tensor_scan(
                    f_sb[:], acst[:], psum_q[:],
                    0.0, mybir.AluOpType.mult, mybir.AluOpType.add)
                nc.sync.dma_start(out_d.ap(), f_sb[:])
    _dedupe_ldweights(nc)
    nc.compile()
    return nc


def _dedupe_ldweights(nc):
    """Drop PE weight loads whose stationary operand matches the previous
    load in the scheduled PE stream (the splitter emits one per matmul)."""
    from concourse import mybir
    dropped = 0
    for bb in nc.m.functions[0].blocks:
        last_key = None
        keep = []
        for inst in bb.instructions:
            if isinstance(inst, mybir.InstLdweights):
                si = inst.sync_info
                a = inst.ins[0]
                key = (str(a.ap), a.offset, str(a.dtype), str(a.memref),
                       str(getattr(inst, "perf_mode", None)))
                if (key == last_key and not (si and (si.on_wait or si.on_update))):
                    dropped += 1
                    continue
                last_key = key
            elif isinstance(inst, (mybir.InstUnconditionalBranch,
                                   mybir.InstCompareAndBranch)):
                last_key = None
            keep.append(inst)
        if len(keep) != len(bb.instructions):
            bb.instructions = keep
    return dropped


def _make_runner(nc):
    """Persistent jitted 8-core runner (mirrors bass2jax.run_bass_via_pjrt)."""
    import jax
    from jax.sharding import Mesh, PartitionSpec
    from jax.experimental.shard_map import shard_map
    from concourse import bass2jax, mybir

    bass2jax.install_neuronx_cc_hook()

    partition_name = (
        nc.partition_id_tensor.name if nc.partition_id_tensor else None
    )
    in_names, out_names, out_avals, zero_shapes = [], [], [], []
    for alloc in nc.m.functions[0].allocations:
        if not isinstance(alloc, mybir.MemoryLocationSet):
            continue
        name = alloc.memorylocations[0].name
        if alloc.kind == "ExternalInput":
            if name != partition_name:
                in_names.append(name)
        elif alloc.kind == "ExternalOutput":
            out_names.append(name)
            shape = tuple(alloc.tensor_shape)
            dtype = mybir.dt.np(alloc.dtype)
            out_avals.append(jax.core.ShapedArray(shape, dtype))
            zero_shapes.append((shape, dtype))
    n_params = len(in_names)
    all_in_names = list(in_names) + list(out_names)
    if partition_name is not None:
        all_in_names.append(partition_name)

    def _body(*args):
        operands = list(args)
        if partition_name is not None:
            operands.append(bass2jax.partition_id_tensor())
        outs = bass2jax._bass_exec_p.bind(
            *operands,
            out_avals=tuple(out_avals),
            in_names=tuple(all_in_names),
            out_names=tuple(out_names),
            lowering_input_output_aliases=(),
            sim_require_finite=True,
            sim_require_nnan=True,
            nc=nc,
        )
        return tuple(outs)

    devices = jax.devices()[:NCORE]
    mesh = Mesh(np.asarray(devices), ("core",))
    n_outs = len(out_names)
    sharded = jax.jit(
        shard_map(_body, mesh=mesh,
                  in_specs=(PartitionSpec("core"),) * (n_params + n_outs),
                  out_specs=(PartitionSpec("core"),) * n_outs,
                  check_rep=False),
        donate_argnums=tuple(range(n_params, n_params + n_outs)),
        keep_unused=True,
    )

    def run(in_maps):
        concat_in = [
            np.concatenate([np.asarray(m[name]) for m in in_maps], axis=0)
            for name in in_names
        ]
        concat_zeros = [
            np.zeros((NCORE * s[0], *s[1:]), d) for (s, d) in zero_shapes
        ]
        out_arrs = sharded(*concat_in, *concat_zeros)
        return [
            {name: np.asarray(out_arrs[i]).reshape(NCORE, *out_avals[i].shape)[c]
             for i, name in enumerate(out_names)}
            for c in range(NCORE)
        ]

    return run


def make_in_maps(prep):
    x8, w1s, t2, wq = prep
    return [dict(x=x8, w1=w1s[k], t2=t2, wq=wq) for k in range(NCORE)]


def _get_runner(wavData, coch_B, coch_A):
    prep = _host_prep(wavData, coch_B, coch_A)
    key = ("v2", _LO_KEEP)
    if key not in _cache:
        nc = _build()
        _cache[key] = _make_runner(nc)
    return _cache[key], make_in_maps(prep)


def kernel(wavData, coch_B, coch_A):
    run, in_maps = _get_runner(wavData, coch_B, coch_A)
    results = run(in_maps)
    out = np.zeros((BS, NCH, NFRM), np.float32)
    for k in range(NCORE):
        F = results[k]["out"]                      # [128, 250]
        out[:, CPC * k + 1: CPC * (k + 1) + 1, :] = \
            F.reshape(CPC, BS, NFRM).transpose(1, 0, 2)
    return out
